# revision 50
# baseline (speedup 1.0000x reference)
"""Fused multi-head attention block (QKV proj -> 16-head attention -> out proj)
for Trainium2, sharded over 8 NeuronCores.

Sharding: batch (4) x head-halves (2) -> each core handles 1 batch element and
8 of the 16 heads. QKV weights are column-sharded per core's heads, the output
projection is row-sharded (Megatron style); the two partial fc outputs per
batch element are summed on the host (cheap fp32 add) so no collectives are
needed.

Per-core device program (all matmuls bf16 inputs, fp32 PSUM accumulation):
  1a. qkT[f, s] = (w_qk^T x^T)[f, s] + b_qk[f]    f = 8 q-heads*64 ++ 8 k-heads*64
      (q side pre-scaled by 1/sqrt(hd) on host)
  1b. v[s, f] = (x w_v)[s, f] + b_v[f], written into per-head [V | ones]
      augmented lhsT blocks for the AV matmul
  2.  attention per head pair: scoresT[k, q] matmuls (contraction hd=64) for
      the two heads packed concurrently on PE rows 0-63 / 64-127
      (tile_position row tiling); exp on ScalarE straight out of PSUM at
      FD=1024 (no max subtraction -- scores are O(1) by construction); AV
      matmul with a ones column producing the softmax denominators in PSUM
      row 64; accumulators fast-evicted to SBUF so PSUM recycles, then
      normalization (reciprocal + DRAM-roundtrip partition broadcast + DVE
      multiply) runs off the critical path. Remaining QKV projections are
      interleaved into stage-2 PE bubbles. attnT is packed by head pair
      (odd heads partition-shifted to 64-127 by DMA).
  3.  fc: attnT pair tiles (K=128) @ w_fc rows for this core's features,
      streamed to DRAM as an fp32 partial.

Container quirks handled here: walrus rejects >~2 sync commands per
instruction (_split_excess_waits hoists excess waits onto injected NoOps)
and custom DVE ops fail codegen (plain nc.vector.reciprocal is used).
"""

import sys

if "/opt/trn_rl_repo" not in sys.path:
    sys.path.insert(0, "/opt/trn_rl_repo")

import numpy as np
import ml_dtypes

_BF16 = ml_dtypes.bfloat16

D = 1024
N_HEAD_CORE = 8  # heads per core
HD = 64
F = N_HEAD_CORE * HD  # 512 features per core
VAUG = HD + 1  # V columns + ones column


def _split_excess_waits(nc, limit=1):
    """This container's walrus rejects instructions carrying more than ~2
    semaphore waits ("Too many sync wait commands"). Hoist excess waits onto
    injected same-engine NoOps placed immediately before the instruction —
    sequential waits are semantically identical to one multi-wait."""
    import bass_rust
    import concourse.mybir as mybir

    n_added = 0
    for fn in nc.m.functions:
        for bb in fn.blocks:
            out = []
            changed = False
            for inst in bb.instructions:
                si = inst.sync_info
                waits = list(si.on_wait) if si and si.on_wait else []
                n_upd = len(si.on_update) if si and si.on_update else 0
                allowed = limit
                if len(waits) > allowed and inst.engine != mybir.EngineType.Unassigned:
                    changed = True
                    extra = waits[: len(waits) - allowed]
                    keep = waits[len(waits) - allowed :]
                    for i in range(0, len(extra), limit):
                        nop = mybir.InstNoOp(
                            name=f"waitsplit_{n_added}", ins=[], outs=[]
                        )
                        n_added += 1
                        nop.engine = inst.engine
                        nop.bass_nofuse = True
                        nop.sync_info = bass_rust.SyncInfo(
                            on_wait=extra[i : i + limit], on_update=[]
                        )
                        out.append(nop)
                    inst.sync_info = bass_rust.SyncInfo(
                        on_wait=keep, on_update=list(si.on_update or [])
                    )
                out.append(inst)
            if changed:
                bb.instructions = out
    return n_added


def build_program(S=2048, split_waits=True, interleave="pairs", reps=1):
    """Emit the per-core Bass program. S = sequence length (parameterized so a
    shrunk version can run under CoreSim)."""
    import concourse.bass as bass
    import concourse.mybir as mybir
    import concourse.tile as tile

    bf = mybir.dt.bfloat16
    f32 = mybir.dt.float32
    Exp = mybir.ActivationFunctionType.Exp

    QC = min(512, S)  # matmul moving-operand chunk along s/q
    n_qc = S // QC
    n_st = S // 128  # 128-row tiles along s
    n_kc = S // 128  # 128-deep k chunks in attention

    nc = bass.Bass("TRN2", target_bir_lowering=False, debug=False, num_devices=1)

    xT_d = nc.dram_tensor("xT", [D, S], bf, kind="ExternalInput").ap()
    wqk_d = nc.dram_tensor("wqk", [D, 2 * F], bf, kind="ExternalInput").ap()
    bqk_d = nc.dram_tensor("bqk", [128, 8], f32, kind="ExternalInput").ap()
    wv_d = nc.dram_tensor("wv", [D, F], bf, kind="ExternalInput").ap()
    bvb_d = nc.dram_tensor("bvb", [128, F], f32, kind="ExternalInput").ap()
    wfc_d = nc.dram_tensor("wfc", [F, D], bf, kind="ExternalInput").ap()
    out_d = nc.dram_tensor("out", [S, D], f32, kind="ExternalOutput").ap()

    out_t = out_d.rearrange("(st p) n -> st p n", p=128)
    wfc_hview = wfc_d.rearrange("(pr p) n -> p pr n", p=128)

    with tile.TileContext(nc) as tc:
        with (
            tc.tile_pool(name="persist", bufs=1) as pp,
            tc.tile_pool(name="evict", bufs=3) as ep,
            tc.tile_pool(name="exp", bufs=4) as xp,
            tc.tile_pool(name="norm", bufs=4) as np_,
            tc.tile_pool(name="avraw", bufs=2) as rp,
            tc.tile_pool(name="dscr", bufs=4, space="DRAM") as dp,
        ):
            xT = pp.tile([128, 8, S], bf)
            wqk = pp.tile([128, 8, 2 * F], bf)
            wv = pp.tile([128, 8, F], bf)
            wfc = pp.tile([128, N_HEAD_CORE // 2, D], bf)
            bqk = pp.tile([128, 8], f32)
            bvb = pp.tile([128, F], f32)
            qkT = pp.tile([128, 8, S], bf)  # mt 0-3: qT rows, 4-7: kT rows
            vaug = pp.tile([128, n_st, N_HEAD_CORE * VAUG], bf)
            # attnT packed by head pair: even head on partitions 0-63, odd
            # head on 64-127 (placed there by a partition-shifting DMA), so
            # the fc contraction runs K=128.
            attnT = pp.tile([128, N_HEAD_CORE // 2, S], bf)

            # Load order tuned for prologue overlap: wv + the first quarter
            # of xT land first so the v-group sweep (emitted before the qk
            # groups) starts ~4us in; wqk trails it, the rest of xT streams
            # behind.
            def load_xt_block(b, nb):
                w = S // nb
                for kt in range(8):
                    nc.sync.dma_start(
                        xT[:, kt, b * w : (b + 1) * w],
                        xT_d[kt * 128 : (kt + 1) * 128, b * w : (b + 1) * w],
                    )

            for kt in range(8):
                nc.sync.dma_start(wv[:, kt, :], wv_d[kt * 128 : (kt + 1) * 128, :])
            nb = 4 if S >= 512 else 1
            load_xt_block(0, nb)
            nc.sync.dma_start(bvb[:], bvb_d[:])
            nc.sync.dma_start(bqk[:], bqk_d[:])
            for kt in range(8):
                nc.sync.dma_start(wqk[:, kt, :], wqk_d[kt * 128 : (kt + 1) * 128, :])
            for b in range(1, nb):
                load_xt_block(b, nb)
            nc.sync.dma_start(wfc[:], wfc_hview)

            for _rep in range(reps):
                # PSUM budget: scores [128, 1024] x2 bufs (4 banks) + AV
                # accumulators OA/OB [65, 512] x1 buf (2 banks) + stage-1
                # accumulation pool psI (2 banks) = 8. psI stays open through
                # stage 2 so the qkT projections for pairs 1-3 interleave into
                # stage-2 PE bubbles (ScalarE is the stage-2 bottleneck).
                with (
                    tc.tile_pool(name="psI", bufs=2, space="PSUM") as psI,
                    tc.tile_pool(name="ps2s", bufs=2, space="PSUM") as ps2s,
                    tc.tile_pool(name="ps2o", bufs=1, space="PSUM") as ps2o,
                ):

                    def emit_qk_group(mt, qc):
                        # stage 1a: qkT[mt] chunk = w_qk^T @ x^T + b
                        acc = psI.tile([128, QC], f32, tag="psI")
                        for kt in range(8):
                            nc.tensor.matmul(
                                acc[:],
                                wqk[:, kt, mt * 128 : (mt + 1) * 128],
                                xT[:, kt, qc * QC : (qc + 1) * QC],
                                start=(kt == 0),
                                stop=(kt == 7),
                            )
                        nc.vector.tensor_scalar_add(
                            qkT[:, mt, qc * QC : (qc + 1) * QC],
                            acc[:],
                            bqk[:, mt : mt + 1],
                        )

                    def emit_v_group(st):
                        # stage 1b: v = x @ w_v + b_v -> per-head [V(64) | ones]
                        acc = psI.tile([128, F], f32, tag="psI")
                        for kt in range(8):
                            nc.tensor.matmul(
                                acc[:],
                                xT[:, kt, st * 128 : (st + 1) * 128],
                                wv[:, kt, :],
                                start=(kt == 0),
                                stop=(kt == 7),
                            )
                        nc.vector.memset(vaug[:, st, :], 1.0)
                        vv = vaug[:, st, :].rearrange("p (h c) -> p h c", c=VAUG)
                        av = acc.rearrange("p (h c) -> p h c", c=HD)
                        bv = bvb.rearrange("p (h c) -> p h c", c=HD)
                        nc.vector.tensor_add(vv[:, :, 0:HD], av[:], bv[:])

                    # Minimal upfront work: pair 0's first q/k chunks + the first
                    # few v tiles. Everything else goes on an ordered job queue
                    # drained one item per stage-2 k-iteration — Tile's
                    # dependency tracking keeps this correct regardless of
                    # pacing; the order just keeps producers ahead of consumers.
                    if interleave == "full":
                        n_up_v = min(6, n_st)
                        emit_qk_group(0, 0)
                        emit_qk_group(4, 0)
                    else:
                        n_up_v = n_st
                        for st in range(n_up_v):
                            emit_v_group(st)
                        for qc in range(n_qc):
                            emit_qk_group(0, qc)
                            emit_qk_group(4, qc)
                    if interleave == "full":
                        for st in range(n_up_v):
                            emit_v_group(st)

                    jobs = []
                    for st in range(n_up_v, n_st):
                        jobs.append(("v", st))
                        # interleave kT chunks of pair 0 so kh arrives before the
                        # scores k-loop reaches it (due at kc = 4*qc)
                        k_idx = len([j for j in jobs if j[0] == "qk4"]) + 1
                        if st % 4 == 1 and k_idx < n_qc:
                            jobs.append(("qk4", k_idx))
                    for qc in range(len([j for j in jobs if j[0] == "qk4"]) + 1, n_qc):
                        jobs.append(("qk4", qc))
                    for qc in range(1, n_qc):
                        jobs.append(("qk0", qc))
                    job_i = [0]

                    def emit_one_early_job():
                        if job_i[0] >= len(jobs):
                            return False
                        j = jobs[job_i[0]]
                        job_i[0] += 1
                        if j[0] == "v":
                            emit_v_group(j[1])
                        elif j[0] == "qk4":
                            emit_qk_group(4, j[1])
                        else:
                            emit_qk_group(0, j[1])
                        return True

                    # qkT for pairs 1-3: one group every `cadence` k-iterations
                    qk_jobs = [
                        (pr, mt, qc)
                        for pr in range(1, N_HEAD_CORE // 2)
                        for mt in (pr, 4 + pr)
                        for qc in range(n_qc)
                    ]
                    if interleave == "none":
                        for _, mt, qc in qk_jobs:
                            emit_qk_group(mt, qc)
                        qk_jobs = []
                    qk_i = [0]

                    def emit_pair_jobs(pair_limit, at_most):
                        n = 0
                        while (
                            qk_i[0] < len(qk_jobs)
                            and qk_jobs[qk_i[0]][0] <= pair_limit
                            and n < at_most
                        ):
                            _, mt, qc = qk_jobs[qk_i[0]]
                            qk_i[0] += 1
                            emit_qk_group(mt, qc)
                            n += 1

                    # ---- stage 2: attention, head pairs packed on the PE ----
                    # The two K=64 scores matmuls of a head pair occupy PE rows
                    # 0-63 and 64-127 (tile_position row tiling, auto-derived
                    # from the qkT slice base partitions) and run concurrently,
                    # writing the two 512-col halves of one [128, 1024] scores
                    # tile; one FD=1024 exp covers both heads.
                    cadence = max(1, (n_qc * n_kc) // (2 * n_qc + 1))
                    it_ctr = [0]
                    for p in range(N_HEAD_CORE // 2):
                        emit_pair_jobs(p, 10**9)  # flush anything this pair needs
                        hA, hB = 2 * p, 2 * p + 1
                        for q5 in range(n_qc):
                            q0 = q5 * QC
                            ovA = ps2o.tile([VAUG, QC], f32, tag="OA")
                            ovB = ps2o.tile([VAUG, QC], f32, tag="OB")
                            for kc in range(n_kc):
                                if not emit_one_early_job():
                                    it_ctr[0] += 1
                                    if it_ctr[0] % cadence == 0:
                                        emit_pair_jobs(p + 1, 1)
                                sc = ps2s.tile([128, 2 * QC], f32, tag="S")
                                nc.tensor.matmul(
                                    sc[:, 0:QC],
                                    qkT[0:HD, 4 + p, kc * 128 : (kc + 1) * 128],
                                    qkT[0:HD, p, q0 : q0 + QC],
                                    start=True,
                                    stop=True,
                                )
                                nc.tensor.matmul(
                                    sc[:, QC : 2 * QC],
                                    qkT[HD:128, 4 + p, kc * 128 : (kc + 1) * 128],
                                    qkT[HD:128, p, q0 : q0 + QC],
                                    start=True,
                                    stop=True,
                                )
                                e = xp.tile([128, 2 * QC], bf, tag="exp")
                                nc.scalar.activation(e[:], sc[:], Exp)
                                nc.tensor.matmul(
                                    ovA[:],
                                    vaug[:, kc, hA * VAUG : (hA + 1) * VAUG],
                                    e[:, 0:QC],
                                    start=(kc == 0),
                                    stop=(kc == n_kc - 1),
                                )
                                nc.tensor.matmul(
                                    ovB[:],
                                    vaug[:, kc, hB * VAUG : (hB + 1) * VAUG],
                                    e[:, QC : 2 * QC],
                                    start=(kc == 0),
                                    stop=(kc == n_kc - 1),
                                )
                            # Fast-evict the accumulators to SBUF (frees PSUM for
                            # the next chunk); the slow normalization (reciprocal
                            # + DRAM-roundtrip partition-broadcast; engines cannot
                            # shift partitions and step-0 DMA sources must be
                            # DRAM) runs off the PSUM critical path.
                            for h, ov in ((hA, ovA), (hB, ovB)):
                                rawsum = rp.tile([VAUG, QC], f32, tag="avraw")
                                nc.vector.tensor_copy(rawsum[:], ov[:])
                                rc = np_.tile([HD + 1, QC], f32, tag="recip")
                                nc.vector.reciprocal(
                                    rc[HD : HD + 1, :], rawsum[HD : HD + 1, :]
                                )
                                dr = dp.tile([1, QC], f32, tag="dscr")
                                nc.sync.dma_start(dr[:], rc[HD : HD + 1, :])
                                bc = np_.tile([HD, QC], f32, tag="bcast")
                                nc.sync.dma_start(
                                    bc[:], dr[0:1, :].to_broadcast((HD, QC))
                                )
                                if h % 2 == 0:
                                    nc.vector.tensor_mul(
                                        attnT[0:HD, p, q0 : q0 + QC],
                                        rawsum[0:HD, :],
                                        bc[:],
                                    )
                                else:
                                    # engines cannot shift partitions: multiply at
                                    # base 0, then DMA onto partitions 64-127
                                    nm = np_.tile([HD, QC], bf, tag="nmul")
                                    nc.vector.tensor_mul(
                                        nm[:], rawsum[0:HD, :], bc[:]
                                    )
                                    nc.sync.dma_start(
                                        attnT[HD:128, p, q0 : q0 + QC], nm[:]
                                    )

                # ---- stage 3: fc partial = attnT^T @ wfc (K=128 per pair) ----
                n_pr = N_HEAD_CORE // 2
                with tc.tile_pool(name="ps3", bufs=4, space="PSUM") as ps3:
                    for st in range(n_st):
                        for oc in range(2):
                            acc = ps3.tile([128, 512], f32, tag="ps3")
                            for pr in range(n_pr):
                                nc.tensor.matmul(
                                    acc[:],
                                    attnT[:, pr, st * 128 : (st + 1) * 128],
                                    wfc[:, pr, oc * 512 : (oc + 1) * 512],
                                    start=(pr == 0),
                                    stop=(pr == n_pr - 1),
                                )
                            o = ep.tile([128, 512], f32, tag="fcout")
                            nc.scalar.copy(o[:], acc[:])
                            nc.sync.dma_start(
                                out_t[st][:, oc * 512 : (oc + 1) * 512], o[:]
                            )

    if split_waits:
        _split_excess_waits(nc)
    return nc


def make_core_inputs(x, w_qkv, b_qkv, w_fc):
    """Shard + lay out host-side inputs for the 8 cores."""
    ins = []
    for core in range(8):
        b, half = core // 2, core % 2
        fsl = slice(half * F, (half + 1) * F)
        w_q = w_qkv[:, 0:D][:, fsl] * np.float32(0.125)
        w_k = w_qkv[:, D : 2 * D][:, fsl]
        w_v = w_qkv[:, 2 * D :][:, fsl]
        b_q = b_qkv[0:D][fsl] * np.float32(0.125)
        b_k = b_qkv[D : 2 * D][fsl]
        b_v = b_qkv[2 * D :][fsl]
        bqk = np.concatenate([b_q, b_k]).astype(np.float32).reshape(8, 128).T
        ins.append(
            {
                "xT": np.ascontiguousarray(x[b].T).astype(_BF16),
                "wqk": np.concatenate([w_q, w_k], axis=1).astype(_BF16),
                "bqk": np.ascontiguousarray(bqk),
                "wv": w_v.astype(_BF16),
                "bvb": np.broadcast_to(b_v.astype(np.float32), (128, F)).copy(),
                "wfc": w_fc[fsl, :].astype(_BF16),
            }
        )
    return ins


_CACHE = {}


def kernel(x, w_qkv, b_qkv, w_fc, b_fc):
    from concourse import bass_utils

    x = np.asarray(x)
    w_qkv = np.asarray(w_qkv)
    b_qkv = np.asarray(b_qkv)
    w_fc = np.asarray(w_fc)
    b_fc = np.asarray(b_fc)
    B, S, _ = x.shape

    if "nc" not in _CACHE:
        _CACHE["nc"] = build_program(S=S)
    nc = _CACHE["nc"]

    in_maps = make_core_inputs(x, w_qkv, b_qkv, w_fc)
    res = bass_utils.run_bass_kernel_spmd(nc, in_maps, core_ids=list(range(8)))
    _CACHE["last_result"] = res

    out = np.empty((B, S, D), dtype=np.float32)
    bfc = b_fc.astype(np.float32)
    for b in range(B):
        out[b] = res.results[2 * b]["out"] + res.results[2 * b + 1]["out"] + bfc
    return out

